# revision 44
# baseline (speedup 1.0000x reference)
"""TRN2 Bass kernel for causal multi-head attention with RoPE.

Problem: B=2, S=2048, HID=2048, NH=16, HD=128 (fp32 in/out).
Sharding: 8 cores = 2 (batch) x 4 (head-groups of 4 heads).
Each core computes q/k/v projections for its 4 heads (column-parallel),
RoPE, causal attention, and a row-parallel partial o_proj; the host sums
the 4 partials per batch.

v2 (all-bf16 dataflow, no DRAM spill):
  - x / Wq / Wk / Wv / Wo converted to bf16 on host: halves HBM traffic
    and makes every matmul 1 cycle/row regardless of tile width.
  - QT/KT live in SBUF as bf16 (2.1MB each) — the v1 DRAM spill round
    trip (16.8MB) and the chunk-0 reload stall are gone.
  - Q/K weight tiles double-buffered (v1 had 7 x ~6.5us stalls at head
    boundaries, each also dropping the PE p-state clock).
  - Softmax sums accumulate on the idle Vector engine (per-tile adds into
    an f32 accumulator) with a single ones-column matmul per (chunk,head)
    instead of one per tile: removes ~26us of PE rows + 160 LDWEIGHTS.
"""
import os
import sys

if "/opt/trn_rl_repo" not in sys.path:
    sys.path.insert(0, "/opt/trn_rl_repo")

import numpy as np
import ml_dtypes

import concourse.bass as bass
import concourse.mybir as mybir
import concourse.tile as tile
from concourse import bacc
from concourse.bass_utils import run_bass_kernel_spmd
from contextlib import ExitStack

P = 128
B, S, HID, NH = 2, 2048, 2048, 16
HD = HID // NH              # 128
H = 4                       # heads per core
DPC = H * HD                # 512 dims per core
KO = HID // P               # 16 contraction chunks
SC = S // 512               # 4 seq chunks of 512
ST = S // P                 # 16 seq tiles of 128
SCALE = 1.0 / float(np.sqrt(HD))

f32 = mybir.dt.float32
f32r = mybir.dt.float32r
bf16 = mybir.dt.bfloat16
fp16 = mybir.dt.float16

_CACHED_NC = None


def build_nc():
    AF = mybir.ActivationFunctionType
    nc = bacc.Bacc(None, target_bir_lowering=False)

    xt = nc.declare_dram_parameter("xt", [P, KO, S], bf16, isOutput=False)
    wq = nc.declare_dram_parameter("wq", [H, P, KO, HD], bf16, isOutput=False)
    wk = nc.declare_dram_parameter("wk", [H, P, KO, HD], bf16, isOutput=False)
    wv = nc.declare_dram_parameter("wv", [P, KO, DPC], bf16, isOutput=False)
    wo = nc.declare_dram_parameter("wo", [P, H, HID], bf16, isOutput=False)
    cosf = nc.declare_dram_parameter("cosf", [P, S], f32, isOutput=False)
    sinf = nc.declare_dram_parameter("sinf", [P, S], f32, isOutput=False)
    bmask = nc.declare_dram_parameter("bmask", [P, H, 512], fp16, isOutput=False)
    # bf16 partials: host sums the 4 head-group partials in f32
    out_p = nc.declare_dram_parameter("out_p", [S, HID], bf16, isOutput=True)

    out3 = out_p.rearrange("(st p) n -> p st n", p=P)

    with tile.TileContext(nc) as tc:
        with ExitStack() as top:
            vpool = top.enter_context(tc.tile_pool(name="vpool", bufs=1))
            qkres = top.enter_context(tc.tile_pool(name="qkres", bufs=1))
            const = top.enter_context(tc.tile_pool(name="const", bufs=1))

            vsb = vpool.tile([P, ST, H, 128], fp16)
            # SBUF-resident transposed Q/K: [d, h, s] in bf16
            qt_sb = qkres.tile([P, H, S], bf16)
            kt_sb = qkres.tile([P, H, S], bf16)

            zb = const.tile([P, 1], f32)
            nc.vector.memset(zb[:], 0.0)
            # warm the scalar-engine exp table so the first attention tile
            # doesn't eat the ACT_TABLE_LOAD latency
            warm = const.tile([P, 1], fp16)
            nc.scalar.activation(warm[:], zb[:], AF.Exp, bias=zb[:], scale=1.0)
            bmt = const.tile([P, H, 512], fp16)

            # ---------------- Phase P: projections ----------------
            with ExitStack() as ctx:
                xpool = ctx.enter_context(tc.tile_pool(name="xp", bufs=1))
                wvpool = ctx.enter_context(tc.tile_pool(name="wvp", bufs=1))
                pp = ctx.enter_context(tc.tile_pool(name="pp", bufs=4, space="PSUM"))

                # per-chunk x tiles + quarter wv tiles: Tile dependencies are
                # tile-granular, so finer tiles let the first V matmuls start
                # after ~2.6MB instead of after the whole stream. wv quarters
                # land first (0.5MB each), then the x chunk halves.
                xsc = [xpool.tile([P, KO, 512], bf16, tag=f"xs{sc}", name=f"xs{sc}")
                       for sc in range(SC)]
                wvq = [wvpool.tile([P, KO // 4, DPC], bf16, tag=f"wv{j}",
                                   name=f"wv{j}") for j in range(4)]
                # DMA bandwidth ramps from ~130GB/s over the first ~20us, so
                # the critical first 2.6MB (x chunk 0 + wv) rides all three
                # queues in parallel; everything else queues behind it.
                cspool = ctx.enter_context(tc.tile_pool(name="cs", bufs=1))
                rtmp = ctx.enter_context(tc.tile_pool(name="rt", bufs=3))
                wpool = ctx.enter_context(tc.tile_pool(name="wqk", bufs=2))
                cosT = cspool.tile([P, S], f32)
                sinT = cspool.tile([P, S], f32)

                # Consumers wait on per-queue DMA completion watermarks taken
                # at their issue point — effectively every dma issued before
                # a matmul gates it. Only the critical first ~2.6MB (x chunk
                # 0 + wv, spread over all three queues) is issued before the
                # first V block; each later chunk is issued right after the
                # previous block's matmuls.
                nc.sync.dma_start(wvq[0][:], wv[:, 0:4])
                nc.scalar.dma_start(wvq[1][:], wv[:, 4:8])
                nc.gpsimd.dma_start(wvq[2][:], wv[:, 8:12])
                nc.sync.dma_start(xsc[0][:, 0:6], xt[:, 0:6, 0:512])
                nc.scalar.dma_start(xsc[0][:, 6:11], xt[:, 6:11, 0:512])
                nc.gpsimd.dma_start(xsc[0][:, 11:16], xt[:, 11:16, 0:512])
                nc.sync.dma_start(wvq[3][:], wv[:, 12:16])

                # V natural layout [s, d]: stationary x tile, moving wv
                # (512-wide => full PE rate)
                def v_block(sc):
                    for st in range(sc * 4, sc * 4 + 4):
                        xc = xsc[st // 4]
                        so = (st % 4) * P
                        ps = pp.tile([P, 512], f32, tag="vproj")
                        for ko in range(KO):
                            wvm = wvq[ko // 4][:, ko % 4]
                            nc.tensor.matmul(
                                ps[:],
                                xc[:, ko, so:so + P],
                                wvm,
                                start=(ko == 0),
                                stop=(ko == KO - 1),
                            )
                        nc.vector.tensor_copy(
                            vsb[:, st],
                            ps.rearrange("p (h d) -> p h d", h=H),
                        )

                for sc in range(SC):
                    v_block(sc)
                    if sc + 1 < SC:
                        nsl = slice((sc + 1) * 512, (sc + 2) * 512)
                        nc.sync.dma_start(xsc[sc + 1][:, 0:8], xt[:, 0:8, nsl])
                        nc.scalar.dma_start(xsc[sc + 1][:, 8:16],
                                            xt[:, 8:16, nsl])
                    if sc == 1:
                        # full-height tables: cos duplicated halves; sin
                        # signed (-sin rows 0:64, +sin rows 64:128) so the
                        # combine is one add
                        nc.gpsimd.dma_start(cosT[:], cosf[:])
                        nc.gpsimd.dma_start(sinT[:], sinf[:])
                        nc.gpsimd.dma_start(bmt[:], bmask[:])

                for w4, dst in ((wq, qt_sb), (wk, kt_sb)):
                    for h in range(H):
                        wt = wpool.tile([P, KO, HD], bf16, tag="w")
                        nc.scalar.dma_start(wt[:], w4[h])
                        for sc in range(SC):
                            ssl = slice(sc * 512, (sc + 1) * 512)
                            ps = pp.tile([P, 512], f32, tag="proj")
                            for ko in range(KO):
                                nc.tensor.matmul(
                                    ps[:],
                                    wt[:, ko],
                                    xsc[sc][:, ko],
                                    start=(ko == 0),
                                    stop=(ko == KO - 1),
                                )
                            # RoPE eviction: partition-shifted reads are
                            # legal only with a PSUM operand, so the two
                            # rotate half-ops read ps directly; the combine
                            # writes bf16 into the resident QT/KT.
                            t0 = rtmp.tile([P, 512], f32, tag="t0")
                            t1 = rtmp.tile([P, 512], f32, tag="t1")
                            nc.vector.tensor_mul(t0[0:64], ps[64:128], sinT[0:64, ssl])
                            nc.vector.tensor_mul(t0[64:128], ps[0:64], sinT[64:128, ssl])
                            nc.vector.tensor_mul(t1[:], ps[:], cosT[:, ssl])
                            nc.vector.tensor_add(dst[:, h, ssl], t1[:], t0[:])

            # ------------- Phase A: attention + interleaved o_proj -------------
            with ExitStack() as ctx:
                ppool = ctx.enter_context(tc.tile_pool(name="ppool", bufs=6))
                smpool = ctx.enter_context(tc.tile_pool(name="smp", bufs=2))
                stage = ctx.enter_context(tc.tile_pool(name="stage", bufs=4))
                aopool = ctx.enter_context(tc.tile_pool(name="ao", bufs=1))
                wopool = ctx.enter_context(tc.tile_pool(name="wop", bufs=1))
                ost = ctx.enter_context(tc.tile_pool(name="ost", bufs=4))
                spsum = ctx.enter_context(tc.tile_pool(name="sps", bufs=2, space="PSUM"))
                opsum = ctx.enter_context(tc.tile_pool(name="ops", bufs=2, space="PSUM"))
                smps = ctx.enter_context(tc.tile_pool(name="smps", bufs=1, space="PSUM"))
                opo = ctx.enter_context(tc.tile_pool(name="opo", bufs=3, space="PSUM"))

                ones_col = const.tile([P, 1], fp16)
                nc.vector.memset(ones_col[:], 1.0)
                # wot's dma is issued after chunk 0's first head (it would
                # gate c0's first matmuls via the queue watermark otherwise)
                wot = wopool.tile([P, H, HID], bf16)

                aot_c = [
                    aopool.tile([P, H, 512], bf16, tag=f"aot{c}", name=f"aot{c}")
                    for c in range(SC)
                ]

                def emit_oproj(cc):
                    for st4 in range(4):
                        st = cc * 4 + st4
                        for nch in range(4):
                            g = st4 * 4 + nch
                            pso = opo.tile([P, 512], f32, tag="po", name="pso")
                            for dc in range(H):
                                nc.tensor.matmul(
                                    pso[:],
                                    aot_c[cc][:, dc, st4 * P:(st4 + 1) * P],
                                    wot[:, dc, nch * 512:(nch + 1) * 512],
                                    start=(dc == 0),
                                    stop=(dc == H - 1),
                                )
                            # PSUM->SBUF eviction split between scalar ACT
                            # and DVE (gpsimd cannot read PSUM); bf16 out
                            # halves the write stream, spread over 3 queues
                            ob = ost.tile([P, 512], bf16, tag="ob", name="ob")
                            if g % 2 == 0:
                                nc.scalar.activation(ob[:], pso[:], AF.Copy)
                            else:
                                nc.vector.tensor_copy(ob[:], pso[:])
                            eng = (nc.sync, nc.gpsimd, nc.scalar)[g % 3]
                            eng.dma_start(
                                out3[:, st, nch * 512:(nch + 1) * 512], ob[:]
                            )

                # Per-head normalize chain (sm matmul -> rcp -> gpsimd
                # broadcast -> DVE mul), pumped ONE STAGE PER TILE of the
                # following head(s). Emitting the whole chain at once parks
                # ops at the head of the in-order DVE/tensor queues waiting
                # on cross-engine inputs and convoys the tile stream; staged,
                # every op is data-ready when its queue reaches it.
                pending = []

                def norm_pump():
                    if not pending:
                        return
                    e = pending[0]
                    s = e["s"]
                    e["s"] += 1
                    if s == 0:
                        e["smps"] = smps.tile([1, 512], f32, tag="smp",
                                              name="smp")
                        nc.tensor.matmul(
                            e["smps"][:], ones_col[:], e["sm"][:],
                            start=True, stop=True,
                        )
                    elif s == 1:
                        e["rcp"] = stage.tile([1, 512], f32, tag="rcp",
                                              name="rcp")
                        nc.vector.reciprocal_approx_fast(
                            e["rcp"][:], e["smps"][:])
                    elif s == 2:
                        e["bc"] = stage.tile([P, 512], f32, tag="bc",
                                             name="bc")
                        nc.gpsimd.partition_broadcast(e["bc"][:], e["rcp"][:])
                    elif s == 3:
                        pass  # one extra slot for the broadcast to land
                    else:
                        nc.vector.tensor_mul(
                            aot_c[e["c"]][:, e["h"]], e["ob"][:], e["bc"][:])
                        pending.pop(0)

                for c in range(SC):
                    qsl = lambda off: slice(c * 512 + off, (c + 1) * 512)
                    nt = 4 * (c + 1)
                    for h in range(H):
                        # finish the chain that owns the recycled ring slot
                        # before reallocating it (only bites in c0's short
                        # 4-tile heads)
                        while len(pending) >= 2:
                            norm_pump()
                        # attn_outT accumulator [d, sq] and DVE softmax-sum
                        # accumulator [k mod 128, sq]
                        ob_ps = opsum.tile([P, 512], f32, tag="obp", name="obp")
                        smacc = smpool.tile([P, 512], fp16, tag="sma", name="sma")
                        # diagonal tiles first: their exp+mask latency hides
                        # behind the dense unmasked tail of this head and the
                        # previous head's stream
                        t_order = list(range(4 * c, nt)) + list(range(0, 4 * c))
                        for ti, t in enumerate(t_order):
                            norm_pump()
                            r = t - 4 * c
                            off = P * max(r, 0)
                            ps = spsum.tile([P, 512], f32, tag="s")
                            nc.tensor.matmul(
                                ps[:, off:512],
                                kt_sb[:, h, t * P:(t + 1) * P],
                                qt_sb[:, h, qsl(off)],
                                start=True,
                                stop=True,
                            )
                            pt = ppool.tile([P, 512], fp16, tag="pt")
                            nc.scalar.activation(
                                pt[:, off:512], ps[:, off:512], AF.Exp,
                                bias=zb[:], scale=SCALE,
                            )
                            if r >= 0:
                                nc.vector.tensor_mul(
                                    pt[:, off:512], pt[:, off:512], bmt[:, r, off:512]
                                )
                            # P@V with V stationary; output is attn_outT [d, sq]
                            nc.tensor.matmul(
                                ob_ps[:, off:512],
                                vsb[:, t, h],
                                pt[:, off:512],
                                start=(ti == 0),
                                stop=(ti == nt - 1),
                            )
                            # softmax-sum partials on DVE (off-PE): first tile
                            # is the r=0 diagonal (off=0, full width), so a
                            # copy initializes the whole accumulator
                            if ti == 0:
                                nc.vector.tensor_copy(smacc[:], pt[:])
                            else:
                                nc.vector.tensor_add(
                                    smacc[:, off:512], smacc[:, off:512],
                                    pt[:, off:512],
                                )
                        pending.append(
                            {"s": 0, "c": c, "h": h, "ob": ob_ps, "sm": smacc}
                        )
                        if c == 0 and h == 0:
                            # issue late so it doesn't gate c0's matmuls
                            nc.gpsimd.dma_start(wot[:], wo[:])

                    # o_proj deferred by one chunk: its aot inputs are then
                    # guaranteed ready, so the PE stream never stalls on the
                    # normalize tail
                    if c > 0:
                        emit_oproj(c - 1)
                while pending:
                    norm_pump()
                emit_oproj(SC - 1)

    nc.compile()
    return nc


def _host_prep(hidden_states, position_ids, Wq, Wk, Wv, Wo):
    """Build the 8 per-core input maps (bf16 weights/activations)."""
    inv_freq = 1.0 / (10000.0 ** (np.arange(0, HD, 2, dtype=np.float32) / HD))
    t = np.arange(S, dtype=np.float32)
    freqs = np.outer(t, inv_freq).astype(np.float32)  # [S, 64]

    bm = np.empty((P, H, 512), dtype=np.float32)
    i = np.arange(P)[:, None, None]
    r = np.arange(H)[None, :, None]
    j = np.arange(512)[None, None, :]
    bm[:] = np.where(i + P * r <= j, 1.0, 0.0)
    bm = bm.astype(np.float16)

    in_maps = []
    per_batch = []
    for b in range(B):
        xT = np.ascontiguousarray(hidden_states[b].T)  # [HID, S]
        xt_sw = np.ascontiguousarray(
            xT.reshape(KO, P, S).transpose(1, 0, 2)
        ).astype(ml_dtypes.bfloat16)  # [P, KO, S]
        fp = freqs[position_ids[b]]  # [S, 64]
        ch = np.cos(fp).T            # [64, S]
        sh = np.sin(fp).T
        cosf = np.ascontiguousarray(np.concatenate([ch, ch], axis=0))   # [128, S]
        sinf = np.ascontiguousarray(np.concatenate([-sh, sh], axis=0))  # signed
        per_batch.append((xt_sw, cosf, sinf))

    for core in range(8):
        b, hg = core // 4, core % 4
        sl = slice(hg * DPC, (hg + 1) * DPC)
        xt_sw, cosf, sinf = per_batch[b]
        wq_sw = np.ascontiguousarray(
            Wq[sl].T.reshape(KO, P, H, HD).transpose(2, 1, 0, 3)
        ).astype(ml_dtypes.bfloat16)  # [H, P, KO, HD]
        wk_sw = np.ascontiguousarray(
            Wk[sl].T.reshape(KO, P, H, HD).transpose(2, 1, 0, 3)
        ).astype(ml_dtypes.bfloat16)
        wv_sw = np.ascontiguousarray(
            Wv[sl].T.reshape(KO, P, DPC).transpose(1, 0, 2)
        ).astype(ml_dtypes.bfloat16)  # [P, KO, DPC]
        wo_sw = np.ascontiguousarray(
            Wo[:, sl].T.reshape(H, HD, HID).transpose(1, 0, 2)
        ).astype(ml_dtypes.bfloat16)  # [P, H, HID]
        in_maps.append({
            "xt": xt_sw, "wq": wq_sw, "wk": wk_sw, "wv": wv_sw, "wo": wo_sw,
            "cosf": cosf, "sinf": sinf, "bmask": bm,
        })
    return in_maps


def kernel(hidden_states, attention_mask, position_ids, Wq, Wk, Wv, Wo,
           _trace=False, _trace_kwargs=None):
    global _CACHED_NC
    hidden_states = np.asarray(hidden_states, dtype=np.float32)
    position_ids = np.asarray(position_ids)
    Wq, Wk, Wv, Wo = (np.asarray(w, dtype=np.float32) for w in (Wq, Wk, Wv, Wo))

    if _CACHED_NC is None:
        _CACHED_NC = build_nc()
    nc = _CACHED_NC

    in_maps = _host_prep(hidden_states, position_ids, Wq, Wk, Wv, Wo)
    res = run_bass_kernel_spmd(
        nc, in_maps, list(range(8)), trace=_trace, **(_trace_kwargs or {})
    )

    out = np.empty((B, S, HID), dtype=np.float32)
    for b in range(B):
        acc = res.results[b * 4]["out_p"].astype(np.float32)
        for hg in range(1, 4):
            acc = acc + res.results[b * 4 + hg]["out_p"].astype(np.float32)
        out[b] = acc
    if _trace:
        return out, res
    return out


# revision 46
# speedup vs baseline: 1.0028x; 1.0028x over previous
"""TRN2 Bass kernel for causal multi-head attention with RoPE.

Problem: B=2, S=2048, HID=2048, NH=16, HD=128 (fp32 in/out).
Sharding: 8 cores = 2 (batch) x 4 (head-groups of 4 heads).
Each core computes q/k/v projections for its 4 heads (column-parallel),
RoPE, causal attention, and a row-parallel partial o_proj; the host sums
the 4 partials per batch.

v2 (all-bf16 dataflow, no DRAM spill):
  - x / Wq / Wk / Wv / Wo converted to bf16 on host: halves HBM traffic
    and makes every matmul 1 cycle/row regardless of tile width.
  - QT/KT live in SBUF as bf16 (2.1MB each) — the v1 DRAM spill round
    trip (16.8MB) and the chunk-0 reload stall are gone.
  - Q/K weight tiles double-buffered (v1 had 7 x ~6.5us stalls at head
    boundaries, each also dropping the PE p-state clock).
  - Softmax sums accumulate on the idle Vector engine (per-tile adds into
    an f32 accumulator) with a single ones-column matmul per (chunk,head)
    instead of one per tile: removes ~26us of PE rows + 160 LDWEIGHTS.
"""
import os
import sys

if "/opt/trn_rl_repo" not in sys.path:
    sys.path.insert(0, "/opt/trn_rl_repo")

import numpy as np
import ml_dtypes

import concourse.bass as bass
import concourse.mybir as mybir
import concourse.tile as tile
from concourse import bacc
from concourse.bass_utils import run_bass_kernel_spmd
from contextlib import ExitStack

P = 128
B, S, HID, NH = 2, 2048, 2048, 16
HD = HID // NH              # 128
H = 4                       # heads per core
DPC = H * HD                # 512 dims per core
KO = HID // P               # 16 contraction chunks
SC = S // 512               # 4 seq chunks of 512
ST = S // P                 # 16 seq tiles of 128
SCALE = 1.0 / float(np.sqrt(HD))

f32 = mybir.dt.float32
f32r = mybir.dt.float32r
bf16 = mybir.dt.bfloat16
fp16 = mybir.dt.float16

_CACHED_NC = None


def build_nc():
    AF = mybir.ActivationFunctionType
    nc = bacc.Bacc(None, target_bir_lowering=False)

    xt = nc.declare_dram_parameter("xt", [P, KO, S], bf16, isOutput=False)
    wq = nc.declare_dram_parameter("wq", [H, P, KO, HD], bf16, isOutput=False)
    wk = nc.declare_dram_parameter("wk", [H, P, KO, HD], bf16, isOutput=False)
    wv = nc.declare_dram_parameter("wv", [P, KO, DPC], bf16, isOutput=False)
    wo = nc.declare_dram_parameter("wo", [P, H, HID], bf16, isOutput=False)
    cosf = nc.declare_dram_parameter("cosf", [P, S], f32, isOutput=False)
    sinf = nc.declare_dram_parameter("sinf", [P, S], f32, isOutput=False)
    bmask = nc.declare_dram_parameter("bmask", [P, H, 512], fp16, isOutput=False)
    # bf16 partials: host sums the 4 head-group partials in f32
    out_p = nc.declare_dram_parameter("out_p", [S, HID], bf16, isOutput=True)

    out3 = out_p.rearrange("(st p) n -> p st n", p=P)

    with tile.TileContext(nc) as tc:
        with ExitStack() as top:
            vpool = top.enter_context(tc.tile_pool(name="vpool", bufs=1))
            qkres = top.enter_context(tc.tile_pool(name="qkres", bufs=1))
            const = top.enter_context(tc.tile_pool(name="const", bufs=1))

            vsb = vpool.tile([P, ST, H, 128], fp16)
            # SBUF-resident transposed Q/K: [d, h, s] in bf16
            qt_sb = qkres.tile([P, H, S], bf16)
            kt_sb = qkres.tile([P, H, S], bf16)

            zb = const.tile([P, 1], f32)
            nc.vector.memset(zb[:], 0.0)
            # warm the scalar-engine exp table so the first attention tile
            # doesn't eat the ACT_TABLE_LOAD latency
            warm = const.tile([P, 1], fp16)
            nc.scalar.activation(warm[:], zb[:], AF.Exp, bias=zb[:], scale=1.0)
            bmt = const.tile([P, H, 512], fp16)

            # ---------------- Phase P: projections ----------------
            with ExitStack() as ctx:
                xpool = ctx.enter_context(tc.tile_pool(name="xp", bufs=1))
                wvpool = ctx.enter_context(tc.tile_pool(name="wvp", bufs=1))
                pp = ctx.enter_context(tc.tile_pool(name="pp", bufs=4, space="PSUM"))

                # per-chunk x tiles + quarter wv tiles: Tile dependencies are
                # tile-granular, so finer tiles let the first V matmuls start
                # after ~2.6MB instead of after the whole stream. wv quarters
                # land first (0.5MB each), then the x chunk halves.
                xsc = [xpool.tile([P, KO, 512], bf16, tag=f"xs{sc}", name=f"xs{sc}")
                       for sc in range(SC)]
                wvq = [wvpool.tile([P, KO // 4, DPC], bf16, tag=f"wv{j}",
                                   name=f"wv{j}") for j in range(4)]
                # DMA bandwidth ramps from ~130GB/s over the first ~20us, so
                # the critical first 2.6MB (x chunk 0 + wv) rides all three
                # queues in parallel; everything else queues behind it.
                cspool = ctx.enter_context(tc.tile_pool(name="cs", bufs=1))
                rtmp = ctx.enter_context(tc.tile_pool(name="rt", bufs=3))
                wpool = ctx.enter_context(tc.tile_pool(name="wqk", bufs=2))
                cosT = cspool.tile([P, S], f32)
                sinT = cspool.tile([P, S], f32)

                # Consumers wait on per-queue DMA completion watermarks taken
                # at their issue point — effectively every dma issued before
                # a matmul gates it, and DMA bandwidth ramps from ~100GB/s
                # over the first ~20us. So the first V seq-chunk loads as
                # four fine [P,KO,128] tiles: the first matmuls are gated on
                # ~1MB instead of the whole 2.6MB critical set.
                xf = [xpool.tile([P, KO, 128], bf16, tag=f"xf{q}",
                                 name=f"xf{q}") for q in range(4)]
                nc.sync.dma_start(wvq[0][:], wv[:, 0:4])
                nc.scalar.dma_start(xf[0][:], xt[:, :, 0:128])
                nc.gpsimd.dma_start(wvq[1][:], wv[:, 4:8])

                def v_group(ps, xc, so, ko_range):
                    for ko in ko_range:
                        nc.tensor.matmul(
                            ps[:],
                            xc[:, ko, so:so + P],
                            wvq[ko // 4][:, ko % 4],
                            start=(ko == 0),
                            stop=(ko == KO - 1),
                        )

                def v_evict(ps, st):
                    nc.vector.tensor_copy(
                        vsb[:, st],
                        ps.rearrange("p (h d) -> p h d", h=H),
                    )

                ps0 = pp.tile([P, 512], f32, tag="vproj", name="ps0")
                v_group(ps0, xf[0], 0, range(0, 8))
                nc.sync.dma_start(wvq[2][:], wv[:, 8:12])
                nc.scalar.dma_start(wvq[3][:], wv[:, 12:16])
                nc.gpsimd.dma_start(xf[1][:], xt[:, :, 128:256])
                v_group(ps0, xf[0], 0, range(8, KO))
                v_evict(ps0, 0)
                nc.sync.dma_start(xf[2][:], xt[:, :, 256:384])
                nc.scalar.dma_start(xf[3][:], xt[:, :, 384:512])
                for q in range(1, 4):
                    psq = pp.tile([P, 512], f32, tag="vproj", name="psq")
                    v_group(psq, xf[q], 0, range(KO))
                    v_evict(psq, q)
                # coarse chunk 0 for the QK projections
                nc.sync.dma_start(xsc[0][:, 0:8], xt[:, 0:8, 0:512])
                nc.scalar.dma_start(xsc[0][:, 8:16], xt[:, 8:16, 0:512])

                # V natural layout [s, d]: stationary x tile, moving wv
                # (512-wide => full PE rate)
                def v_block(sc):
                    for st in range(sc * 4, sc * 4 + 4):
                        xc = xsc[st // 4]
                        so = (st % 4) * P
                        ps = pp.tile([P, 512], f32, tag="vproj", name="ps")
                        for ko in range(KO):
                            wvm = wvq[ko // 4][:, ko % 4]
                            nc.tensor.matmul(
                                ps[:],
                                xc[:, ko, so:so + P],
                                wvm,
                                start=(ko == 0),
                                stop=(ko == KO - 1),
                            )
                        nc.vector.tensor_copy(
                            vsb[:, st],
                            ps.rearrange("p (h d) -> p h d", h=H),
                        )

                for sc in range(1, SC):
                    nsl = slice(sc * 512, (sc + 1) * 512)
                    nc.sync.dma_start(xsc[sc][:, 0:8], xt[:, 0:8, nsl])
                    nc.scalar.dma_start(xsc[sc][:, 8:16], xt[:, 8:16, nsl])
                    if sc == 2:
                        # full-height tables: cos duplicated halves; sin
                        # signed (-sin rows 0:64, +sin rows 64:128) so the
                        # combine is one add
                        nc.gpsimd.dma_start(cosT[:], cosf[:])
                        nc.gpsimd.dma_start(sinT[:], sinf[:])
                        nc.gpsimd.dma_start(bmt[:], bmask[:])
                    v_block(sc)

                for w4, dst in ((wq, qt_sb), (wk, kt_sb)):
                    for h in range(H):
                        wt = wpool.tile([P, KO, HD], bf16, tag="w")
                        nc.scalar.dma_start(wt[:], w4[h])
                        for sc in range(SC):
                            ssl = slice(sc * 512, (sc + 1) * 512)
                            ps = pp.tile([P, 512], f32, tag="proj")
                            for ko in range(KO):
                                nc.tensor.matmul(
                                    ps[:],
                                    wt[:, ko],
                                    xsc[sc][:, ko],
                                    start=(ko == 0),
                                    stop=(ko == KO - 1),
                                )
                            # RoPE eviction: partition-shifted reads are
                            # legal only with a PSUM operand, so the two
                            # rotate half-ops read ps directly; the combine
                            # writes bf16 into the resident QT/KT.
                            t0 = rtmp.tile([P, 512], f32, tag="t0")
                            t1 = rtmp.tile([P, 512], f32, tag="t1")
                            nc.vector.tensor_mul(t0[0:64], ps[64:128], sinT[0:64, ssl])
                            nc.vector.tensor_mul(t0[64:128], ps[0:64], sinT[64:128, ssl])
                            nc.vector.tensor_mul(t1[:], ps[:], cosT[:, ssl])
                            nc.vector.tensor_add(dst[:, h, ssl], t1[:], t0[:])

            # ------------- Phase A: attention + interleaved o_proj -------------
            with ExitStack() as ctx:
                ppool = ctx.enter_context(tc.tile_pool(name="ppool", bufs=6))
                smpool = ctx.enter_context(tc.tile_pool(name="smp", bufs=2))
                stage = ctx.enter_context(tc.tile_pool(name="stage", bufs=4))
                aopool = ctx.enter_context(tc.tile_pool(name="ao", bufs=1))
                wopool = ctx.enter_context(tc.tile_pool(name="wop", bufs=1))
                ost = ctx.enter_context(tc.tile_pool(name="ost", bufs=4))
                spsum = ctx.enter_context(tc.tile_pool(name="sps", bufs=2, space="PSUM"))
                opsum = ctx.enter_context(tc.tile_pool(name="ops", bufs=2, space="PSUM"))
                smps = ctx.enter_context(tc.tile_pool(name="smps", bufs=1, space="PSUM"))
                opo = ctx.enter_context(tc.tile_pool(name="opo", bufs=3, space="PSUM"))

                ones_col = const.tile([P, 1], fp16)
                nc.vector.memset(ones_col[:], 1.0)
                # wot's dma is issued after chunk 0's first head (it would
                # gate c0's first matmuls via the queue watermark otherwise)
                wot = wopool.tile([P, H, HID], bf16)

                aot_c = [
                    aopool.tile([P, H, 512], bf16, tag=f"aot{c}", name=f"aot{c}")
                    for c in range(SC)
                ]

                def emit_oproj(cc):
                    for st4 in range(4):
                        st = cc * 4 + st4
                        for nch in range(4):
                            g = st4 * 4 + nch
                            pso = opo.tile([P, 512], f32, tag="po", name="pso")
                            for dc in range(H):
                                nc.tensor.matmul(
                                    pso[:],
                                    aot_c[cc][:, dc, st4 * P:(st4 + 1) * P],
                                    wot[:, dc, nch * 512:(nch + 1) * 512],
                                    start=(dc == 0),
                                    stop=(dc == H - 1),
                                )
                            # PSUM->SBUF eviction split between scalar ACT
                            # and DVE (gpsimd cannot read PSUM); bf16 out
                            # halves the write stream, spread over 3 queues
                            ob = ost.tile([P, 512], bf16, tag="ob", name="ob")
                            if g % 2 == 0:
                                nc.scalar.activation(ob[:], pso[:], AF.Copy)
                            else:
                                nc.vector.tensor_copy(ob[:], pso[:])
                            eng = (nc.sync, nc.gpsimd, nc.scalar)[g % 3]
                            eng.dma_start(
                                out3[:, st, nch * 512:(nch + 1) * 512], ob[:]
                            )

                # Per-head normalize chain (sm matmul -> rcp -> gpsimd
                # broadcast -> DVE mul), pumped ONE STAGE PER TILE of the
                # following head(s). Emitting the whole chain at once parks
                # ops at the head of the in-order DVE/tensor queues waiting
                # on cross-engine inputs and convoys the tile stream; staged,
                # every op is data-ready when its queue reaches it.
                pending = []

                def norm_pump():
                    if not pending:
                        return
                    e = pending[0]
                    s = e["s"]
                    e["s"] += 1
                    if s == 0:
                        e["smps"] = smps.tile([1, 512], f32, tag="smp",
                                              name="smp")
                        nc.tensor.matmul(
                            e["smps"][:], ones_col[:], e["sm"][:],
                            start=True, stop=True,
                        )
                    elif s == 1:
                        e["rcp"] = stage.tile([1, 512], f32, tag="rcp",
                                              name="rcp")
                        nc.vector.reciprocal_approx_fast(
                            e["rcp"][:], e["smps"][:])
                    elif s == 2:
                        e["bc"] = stage.tile([P, 512], f32, tag="bc",
                                             name="bc")
                        nc.gpsimd.partition_broadcast(e["bc"][:], e["rcp"][:])
                    elif s == 3:
                        pass  # one extra slot for the broadcast to land
                    else:
                        nc.vector.tensor_mul(
                            aot_c[e["c"]][:, e["h"]], e["ob"][:], e["bc"][:])
                        pending.pop(0)

                for c in range(SC):
                    qsl = lambda off: slice(c * 512 + off, (c + 1) * 512)
                    nt = 4 * (c + 1)
                    for h in range(H):
                        # finish the chain that owns the recycled ring slot
                        # before reallocating it (only bites in c0's short
                        # 4-tile heads)
                        while len(pending) >= 2:
                            norm_pump()
                        # attn_outT accumulator [d, sq] and DVE softmax-sum
                        # accumulator [k mod 128, sq]
                        ob_ps = opsum.tile([P, 512], f32, tag="obp", name="obp")
                        smacc = smpool.tile([P, 512], fp16, tag="sma", name="sma")
                        # diagonal tiles first: their exp+mask latency hides
                        # behind the dense unmasked tail of this head and the
                        # previous head's stream
                        t_order = list(range(4 * c, nt)) + list(range(0, 4 * c))
                        for ti, t in enumerate(t_order):
                            norm_pump()
                            r = t - 4 * c
                            off = P * max(r, 0)
                            ps = spsum.tile([P, 512], f32, tag="s")
                            nc.tensor.matmul(
                                ps[:, off:512],
                                kt_sb[:, h, t * P:(t + 1) * P],
                                qt_sb[:, h, qsl(off)],
                                start=True,
                                stop=True,
                            )
                            pt = ppool.tile([P, 512], fp16, tag="pt")
                            nc.scalar.activation(
                                pt[:, off:512], ps[:, off:512], AF.Exp,
                                bias=zb[:], scale=SCALE,
                            )
                            if r >= 0:
                                nc.vector.tensor_mul(
                                    pt[:, off:512], pt[:, off:512], bmt[:, r, off:512]
                                )
                            # P@V with V stationary; output is attn_outT [d, sq]
                            nc.tensor.matmul(
                                ob_ps[:, off:512],
                                vsb[:, t, h],
                                pt[:, off:512],
                                start=(ti == 0),
                                stop=(ti == nt - 1),
                            )
                            # softmax-sum partials on DVE (off-PE): first tile
                            # is the r=0 diagonal (off=0, full width), so a
                            # copy initializes the whole accumulator
                            if ti == 0:
                                nc.vector.tensor_copy(smacc[:], pt[:])
                            else:
                                nc.vector.tensor_add(
                                    smacc[:, off:512], smacc[:, off:512],
                                    pt[:, off:512],
                                )
                        pending.append(
                            {"s": 0, "c": c, "h": h, "ob": ob_ps, "sm": smacc}
                        )
                        if c == 0 and h == 0:
                            # issue late so it doesn't gate c0's matmuls
                            nc.gpsimd.dma_start(wot[:], wo[:])

                    # o_proj deferred by one chunk: its aot inputs are then
                    # guaranteed ready, so the PE stream never stalls on the
                    # normalize tail
                    if c > 0:
                        emit_oproj(c - 1)
                while pending:
                    norm_pump()
                emit_oproj(SC - 1)

    nc.compile()
    return nc


def _host_prep(hidden_states, position_ids, Wq, Wk, Wv, Wo):
    """Build the 8 per-core input maps (bf16 weights/activations)."""
    inv_freq = 1.0 / (10000.0 ** (np.arange(0, HD, 2, dtype=np.float32) / HD))
    t = np.arange(S, dtype=np.float32)
    freqs = np.outer(t, inv_freq).astype(np.float32)  # [S, 64]

    bm = np.empty((P, H, 512), dtype=np.float32)
    i = np.arange(P)[:, None, None]
    r = np.arange(H)[None, :, None]
    j = np.arange(512)[None, None, :]
    bm[:] = np.where(i + P * r <= j, 1.0, 0.0)
    bm = bm.astype(np.float16)

    in_maps = []
    per_batch = []
    for b in range(B):
        xT = np.ascontiguousarray(hidden_states[b].T)  # [HID, S]
        xt_sw = np.ascontiguousarray(
            xT.reshape(KO, P, S).transpose(1, 0, 2)
        ).astype(ml_dtypes.bfloat16)  # [P, KO, S]
        fp = freqs[position_ids[b]]  # [S, 64]
        ch = np.cos(fp).T            # [64, S]
        sh = np.sin(fp).T
        cosf = np.ascontiguousarray(np.concatenate([ch, ch], axis=0))   # [128, S]
        sinf = np.ascontiguousarray(np.concatenate([-sh, sh], axis=0))  # signed
        per_batch.append((xt_sw, cosf, sinf))

    for core in range(8):
        b, hg = core // 4, core % 4
        sl = slice(hg * DPC, (hg + 1) * DPC)
        xt_sw, cosf, sinf = per_batch[b]
        wq_sw = np.ascontiguousarray(
            Wq[sl].T.reshape(KO, P, H, HD).transpose(2, 1, 0, 3)
        ).astype(ml_dtypes.bfloat16)  # [H, P, KO, HD]
        wk_sw = np.ascontiguousarray(
            Wk[sl].T.reshape(KO, P, H, HD).transpose(2, 1, 0, 3)
        ).astype(ml_dtypes.bfloat16)
        wv_sw = np.ascontiguousarray(
            Wv[sl].T.reshape(KO, P, DPC).transpose(1, 0, 2)
        ).astype(ml_dtypes.bfloat16)  # [P, KO, DPC]
        wo_sw = np.ascontiguousarray(
            Wo[:, sl].T.reshape(H, HD, HID).transpose(1, 0, 2)
        ).astype(ml_dtypes.bfloat16)  # [P, H, HID]
        in_maps.append({
            "xt": xt_sw, "wq": wq_sw, "wk": wk_sw, "wv": wv_sw, "wo": wo_sw,
            "cosf": cosf, "sinf": sinf, "bmask": bm,
        })
    return in_maps


def kernel(hidden_states, attention_mask, position_ids, Wq, Wk, Wv, Wo,
           _trace=False, _trace_kwargs=None):
    global _CACHED_NC
    hidden_states = np.asarray(hidden_states, dtype=np.float32)
    position_ids = np.asarray(position_ids)
    Wq, Wk, Wv, Wo = (np.asarray(w, dtype=np.float32) for w in (Wq, Wk, Wv, Wo))

    if _CACHED_NC is None:
        _CACHED_NC = build_nc()
    nc = _CACHED_NC

    in_maps = _host_prep(hidden_states, position_ids, Wq, Wk, Wv, Wo)
    res = run_bass_kernel_spmd(
        nc, in_maps, list(range(8)), trace=_trace, **(_trace_kwargs or {})
    )

    out = np.empty((B, S, HID), dtype=np.float32)
    for b in range(B):
        acc = res.results[b * 4]["out_p"].astype(np.float32)
        for hg in range(1, 4):
            acc = acc + res.results[b * 4 + hg]["out_p"].astype(np.float32)
        out[b] = acc
    if _trace:
        return out, res
    return out


# revision 50
# speedup vs baseline: 1.1755x; 1.1722x over previous
"""TRN2 Bass kernel for causal multi-head attention with RoPE.

Problem: B=2, S=2048, HID=2048, NH=16, HD=128 (fp32 in/out).
Sharding: 8 cores = 2 (batch) x 4 (head-groups of 4 heads).
Each core computes q/k/v projections for its 4 heads (column-parallel),
RoPE, causal attention, and a row-parallel partial o_proj; the host sums
the 4 partials per batch.

Optimizations over the original spill-based fp32r kernel (474-560us ->
~355us unthrottled; measured numbers vary ~1.2x with chip DVFS state):
  - All-bf16/fp16 dataflow: x / Wq / Wk / Wv / Wo host-converted to bf16
    (halves HBM traffic; every matmul 1 cycle/row at any width — fp32r
    drops to 1/4 rate under 256-wide). QT/KT stay resident in SBUF, so
    the 16.8MB DRAM spill round trip and chunk-0 reload stall are gone.
  - Q/K weight tiles double-buffered (was 7 x ~6.5us stalls at head
    boundaries, each also dropping the PE p-state clock).
  - Softmax sums accumulate on the Vector engine in fp16 (2x_1p DVE
    mode) with one ones-column matmul per (chunk,head): removes ~26us
    of PE rows + 160 LDWEIGHTS vs per-tile sum matmuls.
  - The per-head normalize chain (sum-matmul -> reciprocal -> gpsimd
    partition broadcast -> DVE mul) is pumped one stage per tile of the
    following head: emitted all at once it parks at the head of the
    in-order DVE queue and convoys the tile stream.
  - o_proj partials written as bf16 (host sums in f32) over 3 DMA
    queues; PSUM evictions split scalar/vector.
  - DMA issue order matters: consumers wait on per-queue completion
    watermarks taken at their issue point, and DMA bandwidth ramps from
    ~100GB/s over the first ~20us. Only the critical first ~1MB (fine
    [P,KO,128] x tiles + first wv quarter) is issued before the first V
    matmuls; later chunks/tables are issued just ahead of their
    consumers, and wot after chunk 0's first attention head.
"""
import os
import sys

if "/opt/trn_rl_repo" not in sys.path:
    sys.path.insert(0, "/opt/trn_rl_repo")

import numpy as np
import ml_dtypes

import concourse.bass as bass
import concourse.mybir as mybir
import concourse.tile as tile
from concourse import bacc
from concourse.bass_utils import run_bass_kernel_spmd
from contextlib import ExitStack

P = 128
B, S, HID, NH = 2, 2048, 2048, 16
HD = HID // NH              # 128
H = 4                       # heads per core
DPC = H * HD                # 512 dims per core
KO = HID // P               # 16 contraction chunks
SC = S // 512               # 4 seq chunks of 512
ST = S // P                 # 16 seq tiles of 128
SCALE = 1.0 / float(np.sqrt(HD))

f32 = mybir.dt.float32
f32r = mybir.dt.float32r
bf16 = mybir.dt.bfloat16
fp16 = mybir.dt.float16

_CACHED_NC = None


def build_nc():
    AF = mybir.ActivationFunctionType
    nc = bacc.Bacc(None, target_bir_lowering=False)

    xt = nc.declare_dram_parameter("xt", [P, KO, S], bf16, isOutput=False)
    wq = nc.declare_dram_parameter("wq", [H, P, KO, HD], bf16, isOutput=False)
    wk = nc.declare_dram_parameter("wk", [H, P, KO, HD], bf16, isOutput=False)
    wv = nc.declare_dram_parameter("wv", [P, KO, DPC], bf16, isOutput=False)
    wo = nc.declare_dram_parameter("wo", [P, H, HID], bf16, isOutput=False)
    cosf = nc.declare_dram_parameter("cosf", [P, S], f32, isOutput=False)
    sinf = nc.declare_dram_parameter("sinf", [P, S], f32, isOutput=False)
    bmask = nc.declare_dram_parameter("bmask", [P, H, 512], fp16, isOutput=False)
    # bf16 partials: host sums the 4 head-group partials in f32
    out_p = nc.declare_dram_parameter("out_p", [S, HID], bf16, isOutput=True)

    out3 = out_p.rearrange("(st p) n -> p st n", p=P)

    with tile.TileContext(nc) as tc:
        with ExitStack() as top:
            vpool = top.enter_context(tc.tile_pool(name="vpool", bufs=1))
            qkres = top.enter_context(tc.tile_pool(name="qkres", bufs=1))
            const = top.enter_context(tc.tile_pool(name="const", bufs=1))

            vsb = vpool.tile([P, ST, H, 128], fp16)
            # SBUF-resident transposed Q/K, one tile PER HEAD: consumers
            # wait on per-tile write watermarks, so with a single [P,H,S]
            # tile the first attention matmul would gate on the LAST head's
            # RoPE eviction draining the DVE queue (~5us stall)
            qt_h = [qkres.tile([P, S], bf16, tag=f"qt{h}", name=f"qt{h}")
                    for h in range(H)]
            kt_h = [qkres.tile([P, S], bf16, tag=f"kt{h}", name=f"kt{h}")
                    for h in range(H)]

            zb = const.tile([P, 1], f32)
            nc.vector.memset(zb[:], 0.0)
            # warm the scalar-engine exp table so the first attention tile
            # doesn't eat the ACT_TABLE_LOAD latency
            warm = const.tile([P, 1], fp16)
            nc.scalar.activation(warm[:], zb[:], AF.Exp, bias=zb[:], scale=1.0)
            bmt = const.tile([P, H, 512], fp16)

            # ---------------- Phase P: projections ----------------
            with ExitStack() as ctx:
                xpool = ctx.enter_context(tc.tile_pool(name="xp", bufs=1))
                wvpool = ctx.enter_context(tc.tile_pool(name="wvp", bufs=1))
                pp = ctx.enter_context(tc.tile_pool(name="pp", bufs=4, space="PSUM"))

                # per-chunk x tiles + quarter wv tiles: Tile dependencies are
                # tile-granular, so finer tiles let the first V matmuls start
                # after ~2.6MB instead of after the whole stream. wv quarters
                # land first (0.5MB each), then the x chunk halves.
                xsc = [xpool.tile([P, KO, 512], bf16, tag=f"xs{sc}", name=f"xs{sc}")
                       for sc in range(SC)]
                wvq = [wvpool.tile([P, KO // 4, DPC], bf16, tag=f"wv{j}",
                                   name=f"wv{j}") for j in range(4)]
                # DMA bandwidth ramps from ~130GB/s over the first ~20us, so
                # the critical first 2.6MB (x chunk 0 + wv) rides all three
                # queues in parallel; everything else queues behind it.
                cspool = ctx.enter_context(tc.tile_pool(name="cs", bufs=1))
                rtmp = ctx.enter_context(tc.tile_pool(name="rt", bufs=3))
                wpool = ctx.enter_context(tc.tile_pool(name="wqk", bufs=2))
                cosT = cspool.tile([P, S], f32)
                sinT = cspool.tile([P, S], f32)

                # Consumers wait on per-queue DMA completion watermarks taken
                # at their issue point — effectively every dma issued before
                # a matmul gates it, and DMA bandwidth ramps from ~100GB/s
                # over the first ~20us. So the first V seq-chunk loads as
                # four fine [P,KO,128] tiles: the first matmuls are gated on
                # ~1MB instead of the whole 2.6MB critical set.
                xf = [xpool.tile([P, KO, 128], bf16, tag=f"xf{q}",
                                 name=f"xf{q}") for q in range(4)]
                nc.sync.dma_start(wvq[0][:], wv[:, 0:4])
                nc.scalar.dma_start(xf[0][:], xt[:, :, 0:128])
                nc.gpsimd.dma_start(wvq[1][:], wv[:, 4:8])

                def v_group(ps, xc, so, ko_range):
                    for ko in ko_range:
                        nc.tensor.matmul(
                            ps[:],
                            xc[:, ko, so:so + P],
                            wvq[ko // 4][:, ko % 4],
                            start=(ko == 0),
                            stop=(ko == KO - 1),
                        )

                def v_evict(ps, st):
                    nc.vector.tensor_copy(
                        vsb[:, st],
                        ps.rearrange("p (h d) -> p h d", h=H),
                    )

                ps0 = pp.tile([P, 512], f32, tag="vproj", name="ps0")
                v_group(ps0, xf[0], 0, range(0, 8))
                nc.sync.dma_start(wvq[2][:], wv[:, 8:12])
                nc.scalar.dma_start(wvq[3][:], wv[:, 12:16])
                nc.gpsimd.dma_start(xf[1][:], xt[:, :, 128:256])
                v_group(ps0, xf[0], 0, range(8, KO))
                v_evict(ps0, 0)
                nc.sync.dma_start(xf[2][:], xt[:, :, 256:384])
                nc.scalar.dma_start(xf[3][:], xt[:, :, 384:512])
                for q in range(1, 4):
                    psq = pp.tile([P, 512], f32, tag="vproj", name="psq")
                    v_group(psq, xf[q], 0, range(KO))
                    v_evict(psq, q)
                # coarse chunk 0 for the QK projections
                nc.sync.dma_start(xsc[0][:, 0:8], xt[:, 0:8, 0:512])
                nc.scalar.dma_start(xsc[0][:, 8:16], xt[:, 8:16, 0:512])

                # V natural layout [s, d]: stationary x tile, moving wv
                # (512-wide => full PE rate)
                def v_block(sc):
                    for st in range(sc * 4, sc * 4 + 4):
                        xc = xsc[st // 4]
                        so = (st % 4) * P
                        ps = pp.tile([P, 512], f32, tag="vproj", name="ps")
                        for ko in range(KO):
                            wvm = wvq[ko // 4][:, ko % 4]
                            nc.tensor.matmul(
                                ps[:],
                                xc[:, ko, so:so + P],
                                wvm,
                                start=(ko == 0),
                                stop=(ko == KO - 1),
                            )
                        nc.vector.tensor_copy(
                            vsb[:, st],
                            ps.rearrange("p (h d) -> p h d", h=H),
                        )

                for sc in range(1, SC):
                    nsl = slice(sc * 512, (sc + 1) * 512)
                    nc.sync.dma_start(xsc[sc][:, 0:8], xt[:, 0:8, nsl])
                    nc.scalar.dma_start(xsc[sc][:, 8:16], xt[:, 8:16, nsl])
                    if sc == 2:
                        # full-height tables: cos duplicated halves; sin
                        # signed (-sin rows 0:64, +sin rows 64:128) so the
                        # combine is one add
                        nc.gpsimd.dma_start(cosT[:], cosf[:])
                        nc.gpsimd.dma_start(sinT[:], sinf[:])
                        nc.gpsimd.dma_start(bmt[:], bmask[:])
                    v_block(sc)

                for w4, dst_h in ((wq, qt_h), (wk, kt_h)):
                    for h in range(H):
                        wt = wpool.tile([P, KO, HD], bf16, tag="w")
                        nc.scalar.dma_start(wt[:], w4[h])
                        for sc in range(SC):
                            ssl = slice(sc * 512, (sc + 1) * 512)
                            ps = pp.tile([P, 512], f32, tag="proj")
                            for ko in range(KO):
                                nc.tensor.matmul(
                                    ps[:],
                                    wt[:, ko],
                                    xsc[sc][:, ko],
                                    start=(ko == 0),
                                    stop=(ko == KO - 1),
                                )
                            # RoPE eviction: partition-shifted reads are
                            # legal only with a PSUM operand, so the two
                            # rotate half-ops read ps directly; the combine
                            # writes bf16 into the resident QT/KT.
                            t0 = rtmp.tile([P, 512], f32, tag="t0")
                            t1 = rtmp.tile([P, 512], f32, tag="t1")
                            nc.vector.tensor_mul(t0[0:64], ps[64:128], sinT[0:64, ssl])
                            nc.vector.tensor_mul(t0[64:128], ps[0:64], sinT[64:128, ssl])
                            nc.vector.tensor_mul(t1[:], ps[:], cosT[:, ssl])
                            nc.vector.tensor_add(dst_h[h][:, ssl], t1[:], t0[:])

            # ------------- Phase A: attention + interleaved o_proj -------------
            with ExitStack() as ctx:
                ppool = ctx.enter_context(tc.tile_pool(name="ppool", bufs=6))
                smpool = ctx.enter_context(tc.tile_pool(name="smp", bufs=2))
                stage = ctx.enter_context(tc.tile_pool(name="stage", bufs=4))
                aopool = ctx.enter_context(tc.tile_pool(name="ao", bufs=1))
                wopool = ctx.enter_context(tc.tile_pool(name="wop", bufs=1))
                ost = ctx.enter_context(tc.tile_pool(name="ost", bufs=4))
                spsum = ctx.enter_context(tc.tile_pool(name="sps", bufs=2, space="PSUM"))
                opsum = ctx.enter_context(tc.tile_pool(name="ops", bufs=2, space="PSUM"))
                smps = ctx.enter_context(tc.tile_pool(name="smps", bufs=1, space="PSUM"))
                opo = ctx.enter_context(tc.tile_pool(name="opo", bufs=3, space="PSUM"))

                ones_col = const.tile([P, 1], fp16)
                nc.vector.memset(ones_col[:], 1.0)
                # wot's dma is issued after chunk 0's first head (it would
                # gate c0's first matmuls via the queue watermark otherwise)
                wot = wopool.tile([P, H, HID], bf16)

                aot_c = [
                    aopool.tile([P, H, 512], bf16, tag=f"aot{c}", name=f"aot{c}")
                    for c in range(SC)
                ]

                def emit_oproj(cc):
                    for st4 in range(4):
                        st = cc * 4 + st4
                        for nch in range(4):
                            g = st4 * 4 + nch
                            pso = opo.tile([P, 512], f32, tag="po", name="pso")
                            for dc in range(H):
                                nc.tensor.matmul(
                                    pso[:],
                                    aot_c[cc][:, dc, st4 * P:(st4 + 1) * P],
                                    wot[:, dc, nch * 512:(nch + 1) * 512],
                                    start=(dc == 0),
                                    stop=(dc == H - 1),
                                )
                            # PSUM->SBUF eviction split between scalar ACT
                            # and DVE (gpsimd cannot read PSUM); bf16 out
                            # halves the write stream, spread over 3 queues
                            ob = ost.tile([P, 512], bf16, tag="ob", name="ob")
                            if g % 2 == 0:
                                nc.scalar.activation(ob[:], pso[:], AF.Copy)
                            else:
                                nc.vector.tensor_copy(ob[:], pso[:])
                            eng = (nc.sync, nc.gpsimd, nc.scalar)[g % 3]
                            eng.dma_start(
                                out3[:, st, nch * 512:(nch + 1) * 512], ob[:]
                            )

                # Per-head normalize chain (sm matmul -> rcp -> gpsimd
                # broadcast -> DVE mul), pumped ONE STAGE PER TILE of the
                # following head(s). Emitting the whole chain at once parks
                # ops at the head of the in-order DVE/tensor queues waiting
                # on cross-engine inputs and convoys the tile stream; staged,
                # every op is data-ready when its queue reaches it.
                pending = []

                def norm_pump():
                    if not pending:
                        return
                    e = pending[0]
                    s = e["s"]
                    e["s"] += 1
                    if s == 0:
                        e["smps"] = smps.tile([1, 512], f32, tag="smp",
                                              name="smp")
                        nc.tensor.matmul(
                            e["smps"][:], ones_col[:], e["sm"][:],
                            start=True, stop=True,
                        )
                    elif s == 1:
                        e["rcp"] = stage.tile([1, 512], f32, tag="rcp",
                                              name="rcp")
                        nc.vector.reciprocal_approx_fast(
                            e["rcp"][:], e["smps"][:])
                    elif s == 2:
                        e["bc"] = stage.tile([P, 512], f32, tag="bc",
                                             name="bc")
                        nc.gpsimd.partition_broadcast(e["bc"][:], e["rcp"][:])
                    elif s == 3:
                        pass  # one extra slot for the broadcast to land
                    else:
                        nc.vector.tensor_mul(
                            aot_c[e["c"]][:, e["h"]], e["ob"][:], e["bc"][:])
                        pending.pop(0)

                for c in range(SC):
                    qsl = lambda off: slice(c * 512 + off, (c + 1) * 512)
                    nt = 4 * (c + 1)
                    for h in range(H):
                        # finish the chain that owns the recycled ring slot
                        # before reallocating it (only bites in c0's short
                        # 4-tile heads)
                        while len(pending) >= 2:
                            norm_pump()
                        # attn_outT accumulator [d, sq] and DVE softmax-sum
                        # accumulator [k mod 128, sq]
                        ob_ps = opsum.tile([P, 512], f32, tag="obp", name="obp")
                        smacc = smpool.tile([P, 512], fp16, tag="sma", name="sma")
                        # diagonal tiles first: their exp+mask latency hides
                        # behind the dense unmasked tail of this head and the
                        # previous head's stream
                        t_order = list(range(4 * c, nt)) + list(range(0, 4 * c))
                        for ti, t in enumerate(t_order):
                            norm_pump()
                            r = t - 4 * c
                            off = P * max(r, 0)
                            ps = spsum.tile([P, 512], f32, tag="s")
                            nc.tensor.matmul(
                                ps[:, off:512],
                                kt_h[h][:, t * P:(t + 1) * P],
                                qt_h[h][:, qsl(off)],
                                start=True,
                                stop=True,
                            )
                            pt = ppool.tile([P, 512], fp16, tag="pt")
                            nc.scalar.activation(
                                pt[:, off:512], ps[:, off:512], AF.Exp,
                                bias=zb[:], scale=SCALE,
                            )
                            if r >= 0:
                                nc.vector.tensor_mul(
                                    pt[:, off:512], pt[:, off:512], bmt[:, r, off:512]
                                )
                            # P@V with V stationary; output is attn_outT [d, sq]
                            nc.tensor.matmul(
                                ob_ps[:, off:512],
                                vsb[:, t, h],
                                pt[:, off:512],
                                start=(ti == 0),
                                stop=(ti == nt - 1),
                            )
                            # softmax-sum partials on DVE (off-PE): first tile
                            # is the r=0 diagonal (off=0, full width), so a
                            # copy initializes the whole accumulator
                            if ti == 0:
                                nc.vector.tensor_copy(smacc[:], pt[:])
                            else:
                                nc.vector.tensor_add(
                                    smacc[:, off:512], smacc[:, off:512],
                                    pt[:, off:512],
                                )
                        pending.append(
                            {"s": 0, "c": c, "h": h, "ob": ob_ps, "sm": smacc}
                        )
                        if c == 0 and h == 0:
                            # issue late so it doesn't gate c0's matmuls
                            nc.gpsimd.dma_start(wot[:], wo[:])

                    # o_proj deferred by one chunk: its aot inputs are then
                    # guaranteed ready, so the PE stream never stalls on the
                    # normalize tail
                    if c > 0:
                        emit_oproj(c - 1)
                while pending:
                    norm_pump()
                emit_oproj(SC - 1)

    nc.compile()
    return nc


def _host_prep(hidden_states, position_ids, Wq, Wk, Wv, Wo):
    """Build the 8 per-core input maps (bf16 weights/activations)."""
    inv_freq = 1.0 / (10000.0 ** (np.arange(0, HD, 2, dtype=np.float32) / HD))
    t = np.arange(S, dtype=np.float32)
    freqs = np.outer(t, inv_freq).astype(np.float32)  # [S, 64]

    bm = np.empty((P, H, 512), dtype=np.float32)
    i = np.arange(P)[:, None, None]
    r = np.arange(H)[None, :, None]
    j = np.arange(512)[None, None, :]
    bm[:] = np.where(i + P * r <= j, 1.0, 0.0)
    bm = bm.astype(np.float16)

    in_maps = []
    per_batch = []
    for b in range(B):
        xT = np.ascontiguousarray(hidden_states[b].T)  # [HID, S]
        xt_sw = np.ascontiguousarray(
            xT.reshape(KO, P, S).transpose(1, 0, 2)
        ).astype(ml_dtypes.bfloat16)  # [P, KO, S]
        fp = freqs[position_ids[b]]  # [S, 64]
        ch = np.cos(fp).T            # [64, S]
        sh = np.sin(fp).T
        cosf = np.ascontiguousarray(np.concatenate([ch, ch], axis=0))   # [128, S]
        sinf = np.ascontiguousarray(np.concatenate([-sh, sh], axis=0))  # signed
        per_batch.append((xt_sw, cosf, sinf))

    for core in range(8):
        b, hg = core // 4, core % 4
        sl = slice(hg * DPC, (hg + 1) * DPC)
        xt_sw, cosf, sinf = per_batch[b]
        wq_sw = np.ascontiguousarray(
            Wq[sl].T.reshape(KO, P, H, HD).transpose(2, 1, 0, 3)
        ).astype(ml_dtypes.bfloat16)  # [H, P, KO, HD]
        wk_sw = np.ascontiguousarray(
            Wk[sl].T.reshape(KO, P, H, HD).transpose(2, 1, 0, 3)
        ).astype(ml_dtypes.bfloat16)
        wv_sw = np.ascontiguousarray(
            Wv[sl].T.reshape(KO, P, DPC).transpose(1, 0, 2)
        ).astype(ml_dtypes.bfloat16)  # [P, KO, DPC]
        wo_sw = np.ascontiguousarray(
            Wo[:, sl].T.reshape(H, HD, HID).transpose(1, 0, 2)
        ).astype(ml_dtypes.bfloat16)  # [P, H, HID]
        in_maps.append({
            "xt": xt_sw, "wq": wq_sw, "wk": wk_sw, "wv": wv_sw, "wo": wo_sw,
            "cosf": cosf, "sinf": sinf, "bmask": bm,
        })
    return in_maps


def kernel(hidden_states, attention_mask, position_ids, Wq, Wk, Wv, Wo,
           _trace=False, _trace_kwargs=None):
    global _CACHED_NC
    hidden_states = np.asarray(hidden_states, dtype=np.float32)
    position_ids = np.asarray(position_ids)
    Wq, Wk, Wv, Wo = (np.asarray(w, dtype=np.float32) for w in (Wq, Wk, Wv, Wo))

    if _CACHED_NC is None:
        _CACHED_NC = build_nc()
    nc = _CACHED_NC

    in_maps = _host_prep(hidden_states, position_ids, Wq, Wk, Wv, Wo)
    res = run_bass_kernel_spmd(
        nc, in_maps, list(range(8)), trace=_trace, **(_trace_kwargs or {})
    )

    out = np.empty((B, S, HID), dtype=np.float32)
    for b in range(B):
        acc = res.results[b * 4]["out_p"].astype(np.float32)
        for hg in range(1, 4):
            acc = acc + res.results[b * 4 + hg]["out_p"].astype(np.float32)
        out[b] = acc
    if _trace:
        return out, res
    return out
